# revision 6
# baseline (speedup 1.0000x reference)
"""CrossModalAttention kernel for 8x TRN2 NeuronCores (batch data-parallel).

Reference computation (per batch element b, context input is unused):
    qkv = x @ qkv_w + qkv_b            # [N, 3C]
    q, k, v = split(qkv)               # heads H=12, d=64
    attn = softmax(q*scale @ k^T)      # per head, N=1024
    out = (attn @ v) @ proj_w + proj_b # [N, C]

Strategy per core (one batch element each). v5 -- all-fp16 matmuls
(fp8/DoubleRow measured 7e-2 rel err: ~3% per-element quantization does
NOT average out in random-sign contractions), head-pair loop structure,
stall elimination:

  - Head-PAIR scores: head 2p lives in partitions 0:64 and head 2p+1 in
    64:128 of the qkT chunks, so the two scores matmuls (K=64) target
    disjoint PE row-groups via auto tile_position and partially overlap.
    One [128,1024] PSUM tile per (kc, q-half) = even|odd halves, one
    N=1024 exp covers both heads (minimizes ACT instruction count; ACT
    exp is the second-busiest engine at ~110us).
  - v_aug padded to 128 cols/head (64 v dims + ones + 63 zeros): full
    128-col stationary operand re-enables fast weight load; M=65 was
    measured +40%/matmul from serialized LDWEIGHTS. PSUM cost unchanged
    (rows 65:128 of the av banks were dead anyway).
  - Normalization: the whole av [65,1024] is copied PSUM->SBUF first,
    releasing the PSUM bank in ~0.8us instead of holding it through the
    reciprocal + DRAM-broadcast chain (~5us); this killed the ~5us
    per-pair-boundary ACT stalls seen in the v4 trace. Then 1/sums via
    DVE fast reciprocal, partition-broadcast via DRAM round trip, DVE
    multiply into fp16 outT.
  - qkv weight DMAs split so the first columns (m-chunks 0 and 6) land
    first: the first scores matmul unblocks ~4us earlier.
  - proj split 3 ways: A = c 0..2 (after pair 2), B = c 3..4 (+DVE add,
    during pair 5), finish = c5 after the last normalization.
  PSUM (8 banks): sc ring 2x[128,1024] = 4 (also carries qkv/v/proj
  filler groups), av_e+av_o 2x[128,1024] = 4.
"""
import numpy as np

import concourse.bass as bass
import concourse.tile as tile
from concourse import bacc, mybir
from concourse.bass_utils import run_bass_kernel_spmd

DIM = 768
NUM_HEADS = 12
HEAD_DIM = 64
B, N = 8, 1024
P = 128
KC = DIM // P          # 6 contraction chunks of 128 over channels
TC = N // P            # 8 token chunks of 128
HP = NUM_HEADS // 2    # 6 head pairs
VW = 128               # v columns per head: 64 v + 1 ones + 63 zero pad

F32 = mybir.dt.float32
FP16 = mybir.dt.float16


def build_nc(with_qkv_bias: bool, with_proj_bias: bool):
    nc = bacc.Bacc("TRN2", target_bir_lowering=False, debug=False)

    xT_d = nc.dram_tensor("xT", [DIM, N], FP16, kind="ExternalInput")
    wqk_d = nc.dram_tensor("wqk", [DIM, 2 * DIM], FP16, kind="ExternalInput")
    wv_d = nc.dram_tensor("wv", [DIM, DIM], FP16, kind="ExternalInput")
    wproj_d = nc.dram_tensor("wproj", [DIM, DIM], FP16, kind="ExternalInput")
    bqk_d = nc.dram_tensor("bqk", [1, 2 * DIM], F32, kind="ExternalInput")
    bv_d = nc.dram_tensor("bv", [1, DIM], F32, kind="ExternalInput")
    bproj_d = nc.dram_tensor("bproj", [1, DIM], F32, kind="ExternalInput")
    out_d = nc.dram_tensor("out", [N, DIM], FP16, kind="ExternalOutput")

    with tile.TileContext(nc) as tc:
        with (
            tc.tile_pool(name="consts", bufs=1) as consts,
            tc.tile_pool(name="inputs", bufs=1) as in_pool,
            tc.tile_pool(name="qk_sb", bufs=1) as qk_pool,
            tc.tile_pool(name="vaug_sb", bufs=1) as vaug_pool,
            tc.tile_pool(name="outT_sb", bufs=1) as outT_pool,
            tc.tile_pool(name="expT", bufs=10) as exp_pool,
            tc.tile_pool(name="norm", bufs=4) as norm_pool,
            tc.tile_pool(name="rep", bufs=4) as rep_pool,
            tc.tile_pool(name="fin", bufs=4) as fin_pool,
            tc.tile_pool(name="partial", bufs=1) as partial_pool,
            tc.tile_pool(name="dramp", bufs=1, space="DRAM") as dram_pool,
            tc.tile_pool(name="ps_sc", bufs=2, space="PSUM") as ps_sc,
            tc.tile_pool(name="ps_av", bufs=2, space="PSUM") as ps_av,
        ):
            # ---- ACT exp table preload: a dummy exp so the ~2.7us
            # ACT_TABLE_LOAD happens during the input DMA wait ----
            warm_in = consts.tile([1, 16], F32)
            nc.vector.memset(warm_in[:], 0.0)
            warm_out = consts.tile([1, 16], FP16)
            nc.scalar.activation(
                warm_out[:], warm_in[:], mybir.ActivationFunctionType.Exp
            )

            # ---- constants (bias path only; biases are zero here) ----
            if with_qkv_bias or with_proj_bias:
                ones_f32 = consts.tile([1, N], F32)
                nc.vector.memset(ones_f32[:], 1.0)
                ones_row = consts.tile([1, N], FP16)
                nc.vector.tensor_copy(ones_row[:], ones_f32[:])
            if with_qkv_bias:
                bqk_f32 = consts.tile([1, 2 * DIM], F32)
                nc.sync.dma_start(out=bqk_f32[:], in_=bqk_d[:])
                bqk_sb = consts.tile([1, 2 * DIM], FP16)
                nc.vector.tensor_copy(bqk_sb[:], bqk_f32[:])
                bv_f32 = consts.tile([1, DIM], F32)
                nc.sync.dma_start(out=bv_f32[:], in_=bv_d[:])
                bv_sb = consts.tile([1, DIM], FP16)
                nc.vector.tensor_copy(bv_sb[:], bv_f32[:])
            if with_proj_bias:
                bproj_f32 = consts.tile([1, DIM], F32)
                bproj_sb = consts.tile([1, DIM], FP16)

            # ---- input DMAs. wqk columns split so m-chunks 0 (q, cols
            # 0:128) and 6 (k, cols 768:896) land first -> the pair-0
            # scores chain unblocks ~4us earlier. ----
            xT = [in_pool.tile([P, N], FP16, name=f"xT{c}") for c in range(KC)]
            wqk = [
                in_pool.tile([P, 2 * DIM], FP16, name=f"wqk{c}") for c in range(KC)
            ]
            wv = [in_pool.tile([P, DIM], FP16, name=f"wv{c}") for c in range(KC)]
            wproj = [
                in_pool.tile([P, DIM], FP16, name=f"wproj{c}") for c in range(KC)
            ]
            for c in range(KC):
                sl = slice(c * P, (c + 1) * P)
                nc.sync.dma_start(out=xT[c][:], in_=xT_d[sl, :])
                nc.sync.dma_start(out=wqk[c][:, 0:P], in_=wqk_d[sl, 0:P])
                nc.sync.dma_start(
                    out=wqk[c][:, DIM : DIM + P], in_=wqk_d[sl, DIM : DIM + P]
                )
            for c in range(KC):
                sl = slice(c * P, (c + 1) * P)
                nc.sync.dma_start(out=wqk[c][:, P:DIM], in_=wqk_d[sl, P:DIM])
                nc.sync.dma_start(
                    out=wqk[c][:, DIM + P :], in_=wqk_d[sl, DIM + P :]
                )
            for c in range(KC):
                sl = slice(c * P, (c + 1) * P)
                nc.sync.dma_start(out=wv[c][:], in_=wv_d[sl, :])

            # ---- persistent tiles ----
            qkT = [
                qk_pool.tile([P, N], FP16, name=f"qkT{m}") for m in range(2 * KC)
            ]  # m 0-5: q chunk for pair m; 6-11: k chunk for pair m-6.
            #    head even in partitions 0:64, head odd in 64:128.
            v_aug = [
                vaug_pool.tile([P, NUM_HEADS * VW], FP16, name=f"vaug{t}")
                for t in range(TC)
            ]
            outT = [
                outT_pool.tile([P, N], FP16, name=f"outT{p}") for p in range(HP)
            ]
            partials = [
                partial_pool.tile([P, DIM], F32, name=f"pjpart{t}") for t in range(TC)
            ]
            recip_d = dram_pool.tile([NUM_HEADS, N], F32)

            # ---- filler emitters: qkv/v/proj matmul groups the scheduler
            # slots into PE idle gaps while ACT runs exp ----
            def emit_qkT(m, q=None):
                qs = range(2) if q is None else [q]
                width = N if q is None else 512
                ps = ps_sc.tile([P, width], F32, name=f"ps_qk{m}_{qs[0]}", tag="sc")
                msl = slice(m * P, (m + 1) * P)
                for c in range(KC):
                    for qi, qq in enumerate(qs):
                        qsl = slice(qq * 512, (qq + 1) * 512)
                        osl = slice(qi * 512, (qi + 1) * 512)
                        nc.tensor.matmul(
                            ps[:, osl],
                            wqk[c][:, msl],
                            xT[c][:, qsl],
                            start=(c == 0),
                            stop=(c == KC - 1) and not with_qkv_bias,
                        )
                if with_qkv_bias:
                    for qi, qq in enumerate(qs):
                        qsl = slice(qq * 512, (qq + 1) * 512)
                        osl = slice(qi * 512, (qi + 1) * 512)
                        nc.tensor.matmul(
                            ps[:, osl],
                            bqk_sb[:, msl],
                            ones_row[:, qsl],
                            start=False,
                            stop=True,
                        )
                for qi, qq in enumerate(qs):
                    qsl = slice(qq * 512, (qq + 1) * 512)
                    osl = slice(qi * 512, (qi + 1) * 512)
                    nc.vector.tensor_copy(qkT[m][:, qsl], ps[:, osl])

            def emit_v(t):
                ps = ps_sc.tile([P, DIM], F32, name=f"ps_v{t}", tag="sc")
                tsl = slice(t * P, (t + 1) * P)
                for c in range(KC):
                    for nsl in (slice(0, 512), slice(512, DIM)):
                        nc.tensor.matmul(
                            ps[:, nsl],
                            xT[c][:, tsl],
                            wv[c][:, nsl],
                            start=(c == 0),
                            stop=(c == KC - 1) and not with_qkv_bias,
                        )
                if with_qkv_bias:
                    for nsl in (slice(0, 512), slice(512, DIM)):
                        nc.tensor.matmul(
                            ps[:, nsl],
                            ones_row[:, t * P : t * P + P],
                            bv_sb[:, nsl],
                            start=False,
                            stop=True,
                        )
                va3 = v_aug[t][:].rearrange("p (h e) -> p h e", e=VW)
                nc.vector.memset(va3[:, :, 64:65], 1.0)
                nc.vector.memset(va3[:, :, 65:VW], 0.0)
                nc.vector.tensor_copy(
                    va3[:, :, 0:64],
                    ps[:].rearrange("p (h d) -> p h d", d=HEAD_DIM),
                )

            def emit_wproj_dma():
                for c in range(KC):
                    nc.sync.dma_start(
                        out=wproj[c][:], in_=wproj_d[c * P : (c + 1) * P, :]
                    )
                if with_proj_bias:
                    nc.sync.dma_start(out=bproj_f32[:], in_=bproj_d[:])
                    nc.vector.tensor_copy(bproj_sb[:], bproj_f32[:])

            # proj 3-way split: A = c 0..2 (outT pairs 0-2 exist after
            # pair 2), B = c 3..4 accumulated on top via DVE add, finish
            # = c 5 after the last pair's normalization.
            def emit_proj_a(t):
                ps = ps_sc.tile([P, DIM], F32, name=f"pja{t}", tag="sc")
                tsl = slice(t * P, (t + 1) * P)
                for c in range(3):
                    for nsl in (slice(0, 512), slice(512, DIM)):
                        nc.tensor.matmul(
                            ps[:, nsl],
                            outT[c][:, tsl],
                            wproj[c][:, nsl],
                            start=(c == 0),
                            stop=(c == 2),
                        )
                nc.vector.tensor_copy(partials[t][:], ps[:])

            def emit_proj_b(t):
                ps = ps_sc.tile([P, DIM], F32, name=f"pjb{t}", tag="sc")
                tsl = slice(t * P, (t + 1) * P)
                for c in (3, 4):
                    for nsl in (slice(0, 512), slice(512, DIM)):
                        nc.tensor.matmul(
                            ps[:, nsl],
                            outT[c][:, tsl],
                            wproj[c][:, nsl],
                            start=(c == 3),
                            stop=(c == 4),
                        )
                nc.vector.tensor_tensor(
                    out=partials[t][:], in0=ps[:], in1=partials[t][:],
                    op=mybir.AluOpType.add,
                )

            def emit_proj_finish(t):
                ps = ps_sc.tile([P, DIM], F32, name=f"pjf{t}", tag="sc")
                tsl = slice(t * P, (t + 1) * P)
                for nsl in (slice(0, 512), slice(512, DIM)):
                    nc.tensor.matmul(
                        ps[:, nsl],
                        outT[KC - 1][:, tsl],
                        wproj[KC - 1][:, nsl],
                        start=True,
                        stop=not with_proj_bias,
                    )
                if with_proj_bias:
                    for nsl in (slice(0, 512), slice(512, DIM)):
                        nc.tensor.matmul(
                            ps[:, nsl],
                            ones_row[:, t * P : t * P + P],
                            bproj_sb[:, nsl],
                            start=False,
                            stop=True,
                        )
                fin = fin_pool.tile([P, DIM], FP16, name=f"fin{t}", tag="fin")
                nc.vector.tensor_tensor(
                    out=fin[:], in0=ps[:], in1=partials[t][:],
                    op=mybir.AluOpType.add,
                )
                nc.sync.dma_start(out=out_d[tsl, :], in_=fin[:])

            # Filler schedule, keyed (pair, kc-step). Legality: pair p
            # scores need qkT[p] fully and qkT[6+p] half0 by kc0 / half1
            # by kc4; v_aug[kc] is consumed at every pair's step kc.
            fillers = {p: {} for p in range(HP)}
            fillers[0] = {
                0: [(emit_v, (3,))], 1: [(emit_v, (4,))], 2: [(emit_v, (5,))],
                3: [(emit_v, (6,))], 4: [(emit_v, (7,))],
                5: [(emit_qkT, (1,))], 6: [(emit_qkT, (7,))],
            }
            for p in range(1, 5):
                fillers[p] = {1: [(emit_qkT, (p + 1,))], 5: [(emit_qkT, (7 + p,))]}
            fillers[2][6] = [(emit_wproj_dma, ())]
            fillers[3][3] = [(emit_proj_a, (0,))]
            fillers[3][7] = [(emit_proj_a, (1,))]
            for i, t in enumerate(range(2, TC)):
                fillers[4].setdefault(2 + i, []).append((emit_proj_a, (t,)))
            for i, t in enumerate(range(0, 7)):
                fillers[5].setdefault(i, []).append((emit_proj_b, (t,)))

            # ---- prologue: pair 0 operands (dense PE work during the
            # input DMA stream keeps the clock ramping) ----
            emit_qkT(0)
            emit_qkT(6)
            emit_v(0)
            emit_v(1)
            emit_v(2)

            # ---- attention: 6 head pairs; even/odd scores matmuls hit
            # disjoint PE row-groups; one N=1024 exp per (kc, q-half)
            # covers both heads ----
            for p in range(HP):
                qT = qkT[p]
                kT = qkT[HP + p]
                av_e = ps_av.tile([P, N], F32, name=f"av{2 * p}", tag="av")
                av_o = ps_av.tile([P, N], F32, name=f"av{2 * p + 1}", tag="av")
                for kc in range(TC):
                    ksl = slice(kc * P, (kc + 1) * P)
                    for qh in range(2):
                        qsl = slice(qh * 512, (qh + 1) * 512)
                        sc = ps_sc.tile(
                            [P, N], F32, name=f"sc{p}_{kc}_{qh}", tag="sc"
                        )
                        nc.tensor.matmul(
                            sc[:, 0:512], kT[0:64, ksl], qT[0:64, qsl],
                            start=True, stop=True,
                        )
                        nc.tensor.matmul(
                            sc[:, 512:1024], kT[64:128, ksl], qT[64:128, qsl],
                            start=True, stop=True,
                        )
                        eT = exp_pool.tile(
                            [P, N], FP16, name=f"e{p}_{kc}_{qh}", tag="e"
                        )
                        nc.scalar.activation(
                            eT[:], sc[:], mybir.ActivationFunctionType.Exp
                        )
                        nc.tensor.matmul(
                            av_e[:, qsl],
                            v_aug[kc][:, (2 * p) * VW : (2 * p) * VW + VW],
                            eT[:, 0:512],
                            start=(kc == 0), stop=(kc == TC - 1),
                        )
                        nc.tensor.matmul(
                            av_o[:, qsl],
                            v_aug[kc][:, (2 * p + 1) * VW : (2 * p + 1) * VW + VW],
                            eT[:, 512:1024],
                            start=(kc == 0), stop=(kc == TC - 1),
                        )
                    for fn, args in fillers[p].get(kc, []):
                        fn(*args)
                # ---- normalization. The full av [65,1024] goes to SBUF
                # first so the PSUM banks free up immediately (the DRAM
                # broadcast chain otherwise held them ~5us and stalled
                # the next pair); then 1/sums, partition-broadcast via
                # DRAM, DVE multiply into fp16 outT. ----
                for i, av in ((0, av_e), (1, av_o)):
                    h = 2 * p + i
                    hrow = slice(i * 64, (i + 1) * 64)
                    sums_t = norm_pool.tile([1, N], F32, name=f"sums{h}", tag="sums")
                    recip_t = norm_pool.tile(
                        [1, N], F32, name=f"recip{h}", tag="recip"
                    )
                    nc.vector.tensor_copy(sums_t[:], av[64:65, :])
                    nc.vector.reciprocal_approx_fast(out=recip_t[:], in_=sums_t[:])
                    nc.sync.dma_start(out=recip_d[h : h + 1, :], in_=recip_t[:])
                    rep = rep_pool.tile([64, N], F32, name=f"rep{h}", tag="rep")
                    nc.sync.dma_start(
                        out=rep[:],
                        in_=recip_d[h : h + 1, :].to_broadcast([64, N]),
                    )
                    nc.vector.tensor_tensor(
                        out=outT[p][hrow, :],
                        in0=av[0:64, :],
                        in1=rep[:],
                        op=mybir.AluOpType.mult,
                    )

            # ---- epilogue: last proj-B, then the c=5 finishes ----
            emit_proj_b(7)
            for t in range(TC):
                emit_proj_finish(t)

    nc.compile()
    return nc


def prep_in_maps(inputs):
    x = np.asarray(inputs["x"], dtype=np.float32)
    qkv_w = np.asarray(inputs["qkv_w"], dtype=np.float32)
    qkv_b = np.asarray(inputs["qkv_b"], dtype=np.float32)
    proj_w = np.asarray(inputs["proj_w"], dtype=np.float32)
    proj_b = np.asarray(inputs["proj_b"], dtype=np.float32)
    # context is unused by the reference layer.

    scale = HEAD_DIM ** -0.5
    wqk = qkv_w[:, : 2 * DIM].copy()
    wqk[:, :DIM] *= scale
    bqk = qkv_b[: 2 * DIM].copy()
    bqk[:DIM] *= scale

    base = {
        "wqk": wqk.astype(np.float16),
        "wv": np.ascontiguousarray(qkv_w[:, 2 * DIM :]).astype(np.float16),
        "wproj": proj_w.astype(np.float16),
        "bqk": bqk.reshape(1, -1).astype(np.float32),
        "bv": qkv_b[2 * DIM :].reshape(1, -1).astype(np.float32),
        "bproj": proj_b.reshape(1, -1).astype(np.float32),
    }
    in_maps = [
        {**base, "xT": np.ascontiguousarray(x[b].T).astype(np.float16)}
        for b in range(B)
    ]
    with_qkv_bias = bool(np.any(qkv_b))
    with_proj_bias = bool(np.any(proj_b))
    return in_maps, with_qkv_bias, with_proj_bias


_NC_CACHE = {}


def kernel(**inputs) -> np.ndarray:
    in_maps, with_qkv_bias, with_proj_bias = prep_in_maps(inputs)
    key = (with_qkv_bias, with_proj_bias)
    if key not in _NC_CACHE:
        _NC_CACHE[key] = build_nc(*key)
    nc = _NC_CACHE[key]
    res = run_bass_kernel_spmd(nc, in_maps, list(range(B)))
    out = np.stack([res.results[b]["out"] for b in range(B)], axis=0)
    return out.astype(np.float32)
